# revision 1
# baseline (speedup 1.0000x reference)
"""CartBonded whole-pose scoring on 8 Trainium2 NeuronCores.

Sharding (pose-major, per sharding hint): core c owns poses [8c, 8c+8).
Host: buckets term lists by pose (stable sort), pads each (pose, type)
bucket to a fixed [128, F] tile, expands per-term spring constants
K = global_params[param_idx] ("tuples + their params"), and ships a
per-core coords table [8*16384, 4] f32.
The host pass also materializes per-term atom coords in tile layout
(the multi-index indirect-DMA path mis-orders indices on TRN2 HW, so
the gather rides the same host permutation that shards the term lists).
Device: per (pose, type) tile — stream coord/param tiles from HBM,
DVE/ACT term math, fused per-pose segment sum via scalar_tensor_tensor
accum_out; final cross-partition reduce via a ones-vector matmul on PE.
"""

import numpy as np

N_POSES = 64
MAX_ATOMS = 16384
N_CORES = 8
PP = N_POSES // N_CORES  # poses per core
P = 128
EPS = 1e-12
PI = float(np.pi)

_BUILD_CACHE = {}


# ----------------------------------------------------------------- host prep
def _prep_type(atoms, param_idx, x0, K_table, arity):
    """Bucket terms by pose, pad to [N_POSES, arity, P, F] tiles.

    Returns F, idx [N_POSES, arity, P, F] int32 (core-local flat atom row),
    K [N_POSES, P, F] f32 (0 on pads), x0 [N_POSES, P, F] f32.
    """
    n = atoms.shape[0]
    pose = (atoms[:, 0] // MAX_ATOMS).astype(np.int64)
    order = np.argsort(pose, kind="stable")
    pose_s = pose[order]
    atoms_s = atoms[order].astype(np.int64)
    x0_s = x0[order]
    K_s = K_table[param_idx[order]]

    counts = np.bincount(pose, minlength=N_POSES)
    F = -(-int(counts.max()) // P)  # ceil(max/P)
    F = -(-F // 4) * 4  # multiple of 4
    starts = np.zeros(N_POSES + 1, np.int64)
    np.cumsum(counts, out=starts[1:])
    r = np.arange(n, dtype=np.int64) - starts[pose_s]
    part = (r // F).astype(np.int64)
    free = (r % F).astype(np.int64)
    assert part.max() < P

    local = atoms_s - (pose_s * MAX_ATOMS)[:, None]
    corelocal = (local + ((pose_s % PP) * MAX_ATOMS)[:, None]).astype(np.int32)

    idx = np.zeros((N_POSES, arity, P, F), np.int32)
    idx[pose_s, :, part, free] = corelocal
    Kp = np.zeros((N_POSES, P, F), np.float32)
    Kp[pose_s, part, free] = K_s
    x0p = np.zeros((N_POSES, P, F), np.float32)
    x0p[pose_s, part, free] = x0_s
    return F, idx, Kp, x0p


# --------------------------------------------------------------- device build
def _build(Fb, Fa, Ft):
    key = (Fb, Fa, Ft)
    if key in _BUILD_CACHE:
        return _BUILD_CACHE[key]

    import concourse.bass as bass
    import concourse.tile as tile
    from concourse import bacc, mybir

    dt = mybir.dt
    f32 = dt.float32
    Act = mybir.ActivationFunctionType
    Op = mybir.AluOpType

    nc = bacc.Bacc("TRN2", target_bir_lowering=False, debug=False,
                   num_devices=N_CORES)

    bidx = nc.dram_tensor("bg", [PP, 2, P, Fb, 4], f32,
                          kind="ExternalInput").ap()
    bK = nc.dram_tensor("bK", [PP, P, Fb], f32, kind="ExternalInput").ap()
    bx0 = nc.dram_tensor("bx0", [PP, P, Fb], f32, kind="ExternalInput").ap()
    aidx = nc.dram_tensor("ag", [PP, 3, P, Fa, 4], f32,
                          kind="ExternalInput").ap()
    aK = nc.dram_tensor("aK", [PP, P, Fa], f32, kind="ExternalInput").ap()
    ax0 = nc.dram_tensor("ax0", [PP, P, Fa], f32, kind="ExternalInput").ap()
    tidx = nc.dram_tensor("tg", [PP, 4, P, Ft, 4], f32,
                          kind="ExternalInput").ap()
    tK = nc.dram_tensor("tK", [PP, P, Ft], f32, kind="ExternalInput").ap()
    tx0 = nc.dram_tensor("tx0", [PP, P, Ft], f32, kind="ExternalInput").ap()
    out = nc.dram_tensor("out", [1, PP], f32, kind="ExternalOutput").ap()

    for v in (EPS, -PI):
        t = nc.alloc_sbuf_tensor(f"constf32-{v}", [P, 1], f32)
        nc.gpsimd.memset(t.ap(), v)
        nc.const_aps.aps[(f32, v)] = t.ap()
    nc.all_engine_barrier()

    from contextlib import ExitStack

    with tile.TileContext(nc) as tc, ExitStack() as ctx:
        pers = ctx.enter_context(tc.tile_pool(name="pers", bufs=1))
        gpool = ctx.enter_context(tc.tile_pool(name="g", bufs=2))
        ipool = ctx.enter_context(tc.tile_pool(name="i", bufs=2))
        xkpool = ctx.enter_context(tc.tile_pool(name="xk", bufs=2))
        tp = ctx.enter_context(tc.tile_pool(name="tmp", bufs=1))
        psum = ctx.enter_context(tc.tile_pool(name="ps", bufs=1, space="PSUM"))

        partials = pers.tile([P, PP * 3], f32)

        V = nc.vector

        def gather(g_dram, pose, slot, F):
            g = gpool.tile([P, F, 4], f32, tag=f"g{slot}", name=f"g{slot}")
            nc.gpsimd.dma_start(g[:], g_dram[pose, slot])
            return g

        def loadxk(K_dram, x0_dram, pose, F):
            K = xkpool.tile([P, F], f32, tag="K", name="Kt")
            nc.sync.dma_start(K[:], K_dram[pose])
            X0 = xkpool.tile([P, F], f32, tag="X0", name="X0t")
            nc.sync.dma_start(X0[:], x0_dram[pose])
            return K, X0

        def T(tag, F):
            return tp.tile([P, F], f32, tag=tag, name=tag)

        def sub(o, a, b):
            V.tensor_tensor(out=o[:], in0=a, in1=b, op=Op.subtract)
            return o

        def mul(o, a, b):
            V.tensor_tensor(out=o[:], in0=a, in1=b, op=Op.mult)
            return o

        def add(o, a, b):
            V.tensor_tensor(out=o[:], in0=a, in1=b, op=Op.add)
            return o

        def diff3(pref, gA, gB, F):
            return [sub(T(f"{pref}{c}", F), gA[:, :, c], gB[:, :, c])
                    for c in range(3)]

        def cross(pref, u, v, F):
            # (u x v)_c = u[c+1]*v[c+2] - u[c+2]*v[c+1] (indices mod 3)
            res = []
            for c in range(3):
                ta = mul(T("cta", F), u[(c + 1) % 3][:], v[(c + 2) % 3][:])
                tb = mul(T("ctb", F), u[(c + 2) % 3][:], v[(c + 1) % 3][:])
                res.append(sub(T(f"{pref}{c}", F), ta[:], tb[:]))
            return res

        def dot(tag, u, v, F):
            acc = mul(T(tag, F), u[0][:], v[0][:])
            for c in (1, 2):
                ta = mul(T("dta", F), u[c][:], v[c][:])
                add(acc, acc[:], ta[:])
            return acc

        def norm2(tag, u, F):
            acc = mul(T(tag, F), u[0][:], u[0][:])
            for c in (1, 2):
                ta = mul(T("dta", F), u[c][:], u[c][:])
                add(acc, acc[:], ta[:])
            return acc

        def emit_energy(pre, K, col, F):
            # partials[:, col] = sum_free((pre + 1?) ... ) handled by caller
            e = T("e", F)
            V.scalar_tensor_tensor(
                out=e[:], in0=pre[:], scalar=0.0, in1=K[:],
                op0=Op.add, op1=Op.mult,
                accum_out=partials[:, col:col + 1])

        def bond(pose):
            g0 = gather(bidx, pose, 0, Fb)
            g1 = gather(bidx, pose, 1, Fb)
            K, X0 = loadxk(bK, bx0, pose, Fb)
            d = diff3("bd", g0, g1, Fb)
            D2 = norm2("D2", d, Fb)
            dd = T("dd", Fb)
            nc.scalar.activation(dd[:], D2[:], Act.Sqrt, bias=EPS)
            sub(dd, dd[:], X0[:])
            sq = mul(T("sq", Fb), dd[:], dd[:])
            emit_energy(sq, K, pose * 3 + 0, Fb)

        def angle(pose):
            g0 = gather(aidx, pose, 0, Fa)
            g1 = gather(aidx, pose, 1, Fa)
            g2 = gather(aidx, pose, 2, Fa)
            K, X0 = loadxk(aK, ax0, pose, Fa)
            u = diff3("au", g0, g1, Fa)
            v = diff3("av", g2, g1, Fa)
            cx = cross("acx", u, v, Fa)
            S = norm2("S", cx, Fa)
            x = dot("xx", u, v, Fa)
            y = T("yy", Fa)
            nc.scalar.activation(y[:], S[:], Act.Sqrt, bias=EPS)
            ax = T("ax", Fa)
            nc.scalar.activation(ax[:], x[:], Act.Abs)
            a = T("aa", Fa)
            V.tensor_tensor(out=a[:], in0=ax[:], in1=y[:], op=Op.min)
            b = T("bb", Fa)
            V.tensor_tensor(out=b[:], in0=ax[:], in1=y[:], op=Op.max)
            ib = T("ib", Fa)
            V.reciprocal_approx_fast(ib[:], b[:])
            t = mul(T("tt", Fa), a[:], ib[:])
            phi = T("phi", Fa)
            nc.scalar.activation(phi[:], t[:], Act.Arctan)
            sgn = T("sgn", Fa)
            nc.scalar.activation(sgn[:], x[:], Act.Sign)
            m = T("mm", Fa)
            V.tensor_tensor(out=m[:], in0=ax[:], in1=y[:], op=Op.is_le)
            s1 = T("s1", Fa)
            V.tensor_scalar(out=s1[:], in0=m[:], scalar1=-2.0, scalar2=1.0,
                            op0=Op.mult, op1=Op.add)  # 1-2m
            G = mul(T("GG", Fa), phi[:], s1[:])
            w = T("ww", Fa)
            V.tensor_scalar(out=w[:], in0=m[:], scalar1=PI / 2,
                            scalar2=-PI / 2, op0=Op.mult, op1=Op.add)
            add(G, G[:], w[:])
            sG = mul(T("sG", Fa), sgn[:], G[:])
            x0pp = T("x0pp", Fa)
            V.tensor_scalar(out=x0pp[:], in0=X0[:], scalar1=-1.0,
                            scalar2=PI / 2, op0=Op.mult, op1=Op.add)
            dd = add(T("dd", Fa), sG[:], x0pp[:])
            sq = mul(T("sq", Fa), dd[:], dd[:])
            emit_energy(sq, K, pose * 3 + 1, Fa)

        def torsion(pose):
            g0 = gather(tidx, pose, 0, Ft)
            g1 = gather(tidx, pose, 1, Ft)
            g2 = gather(tidx, pose, 2, Ft)
            g3 = gather(tidx, pose, 3, Ft)
            K, X0 = loadxk(tK, tx0, pose, Ft)
            b1 = diff3("tb1", g1, g0, Ft)
            b2 = diff3("tb2", g2, g1, Ft)
            b3 = diff3("tb3", g3, g2, Ft)
            n1 = cross("tn1", b1, b2, Ft)
            n2 = cross("tn2", b2, b3, Ft)
            S2 = norm2("S2", b2, Ft)
            r = T("rr", Ft)
            nc.scalar.activation(r[:], S2[:], Act.Sqrt, bias=EPS)
            ir = T("ir", Ft)
            V.reciprocal_approx_fast(ir[:], r[:])
            b2n = [mul(T(f"e2{c}", Ft), b2[c][:], ir[:]) for c in range(3)]
            m1 = cross("tm1", n1, b2n, Ft)
            A = dot("AA", m1, n2, Ft)
            B = dot("BB", n1, n2, Ft)
            R2 = norm2_2(A, B, Ft)
            R = T("RR", Ft)
            nc.scalar.activation(R[:], R2[:], Act.Sqrt, bias=EPS)
            iR = T("iR", Ft)
            V.reciprocal_approx_fast(iR[:], R[:])
            c = mul(T("cc", Ft), B[:], iR[:])
            s = mul(T("ss", Ft), A[:], iR[:])
            c2 = mul(T("c2", Ft), c[:], c[:])
            tq = T("tq", Ft)
            V.tensor_scalar(out=tq[:], in0=c2[:], scalar1=4.0, scalar2=-3.0,
                            op0=Op.mult, op1=Op.add)
            c3 = mul(T("c3", Ft), c[:], tq[:])
            s2q = mul(T("s2q", Ft), s[:], s[:])
            t2 = T("t2", Ft)
            V.tensor_scalar(out=t2[:], in0=s2q[:], scalar1=-4.0, scalar2=3.0,
                            op0=Op.mult, op1=Op.add)
            s3 = mul(T("s3", Ft), s[:], t2[:])
            # cos(x0) = sin(y2), y2 = range-reduced (pi/2 - x0)
            y1 = T("y1", Ft)
            V.tensor_scalar(out=y1[:], in0=X0[:], scalar1=-1.0,
                            scalar2=PI / 2, op0=Op.mult, op1=Op.add)
            mm = T("mm", Ft)
            V.tensor_scalar(out=mm[:], in0=y1[:], scalar1=-PI, scalar2=None,
                            op0=Op.is_lt)
            y2 = T("y2", Ft)
            V.scalar_tensor_tensor(out=y2[:], in0=mm[:], scalar=2 * PI,
                                   in1=y1[:], op0=Op.mult, op1=Op.add)
            cx0 = T("cx0", Ft)
            nc.scalar.activation(cx0[:], y2[:], Act.Sin)
            # sin(x0) = -sin(x0 - pi)
            sinz = T("sinz", Ft)
            nc.scalar.activation(sinz[:], X0[:], Act.Sin, bias=-PI)
            w = mul(T("ww", Ft), c3[:], cx0[:])
            v = mul(T("vv", Ft), s3[:], sinz[:])
            u = sub(T("uu", Ft), w[:], v[:])  # c3*cos(x0) + s3*sin(x0)
            e = T("e", Ft)
            V.scalar_tensor_tensor(
                out=e[:], in0=u[:], scalar=1.0, in1=K[:],
                op0=Op.add, op1=Op.mult,
                accum_out=partials[:, (pose * 3 + 2):(pose * 3 + 3)])

        def norm2_2(A, B, F):
            a2 = mul(T("a2", F), A[:], A[:])
            b2_ = mul(T("dta", F), B[:], B[:])
            return add(a2, a2[:], b2_[:])

        for pose in range(PP):
            bond(pose)
            angle(pose)
            torsion(pose)

        ones = pers.tile([P, 1], f32)
        V.memset(ones[:], 1.0)
        ps = psum.tile([1, PP * 3], f32)
        nc.tensor.matmul(out=ps[:], lhsT=ones[:], rhs=partials[:],
                         start=True, stop=True)
        psc = pers.tile([1, PP * 3], f32)
        V.tensor_copy(out=psc[:], in_=ps[:])
        s8 = pers.tile([1, PP], f32)
        V.tensor_tensor(out=s8[:], in0=psc[0:1, 0:PP * 3:3],
                        in1=psc[0:1, 1:PP * 3:3], op=Op.add)
        V.tensor_tensor(out=s8[:], in0=s8[:], in1=psc[0:1, 2:PP * 3:3],
                        op=Op.add)
        nc.sync.dma_start(out[:], s8[:])

    nc.compile()
    _BUILD_CACHE[key] = nc
    return nc


# ---------------------------------------------------------------------- main
def kernel(coords, global_params, bond_x0, angle_x0, tor_x0,
           bond_atoms, bond_param_idx, angle_atoms, angle_param_idx,
           tor_atoms, tor_param_idx, _trace=False):
    coords = np.asarray(coords, dtype=np.float32)
    K_table = np.asarray(global_params, dtype=np.float32)[:, 0]

    Fb, bidx, bK, bx0 = _prep_type(np.asarray(bond_atoms),
                                   np.asarray(bond_param_idx),
                                   np.asarray(bond_x0, np.float32),
                                   K_table, 2)
    Fa, aidx, aK, ax0 = _prep_type(np.asarray(angle_atoms),
                                   np.asarray(angle_param_idx),
                                   np.asarray(angle_x0, np.float32),
                                   K_table, 3)
    Ft, tidx, tK, tx0 = _prep_type(np.asarray(tor_atoms),
                                   np.asarray(tor_param_idx),
                                   np.asarray(tor_x0, np.float32),
                                   K_table, 4)

    nc = _build(Fb, Fa, Ft)

    ctab_all = np.zeros((N_CORES, PP * MAX_ATOMS, 4), np.float32)
    ctab_all[:, :, :3] = coords.reshape(N_CORES, PP * MAX_ATOMS, 3)

    in_maps = []
    for c in range(N_CORES):
        lo, hi = c * PP, (c + 1) * PP
        in_maps.append({
            "bg": ctab_all[c][bidx[lo:hi]], "bK": bK[lo:hi], "bx0": bx0[lo:hi],
            "ag": ctab_all[c][aidx[lo:hi]], "aK": aK[lo:hi], "ax0": ax0[lo:hi],
            "tg": ctab_all[c][tidx[lo:hi]], "tK": tK[lo:hi], "tx0": tx0[lo:hi],
        })

    from concourse.bass_utils import run_bass_kernel_spmd
    res = run_bass_kernel_spmd(nc, in_maps, list(range(N_CORES)),
                               trace=_trace)
    out = np.concatenate([res.results[c]["out"][0] for c in range(N_CORES)])
    if _trace:
        kernel._last_result = res
    return out.astype(np.float32)



# revision 3
# speedup vs baseline: 2.5529x; 2.5529x over previous
"""CartBonded whole-pose scoring on 8 Trainium2 NeuronCores.

Sharding (pose-major, per sharding hint): core c owns poses [8c, 8c+8).
Host: buckets term lists by pose (stable sort), pads each (pose, type)
bucket to fixed [128, F] tiles, expands per-term spring constants
K = global_params[param_idx], and materializes per-term atom coords in
tile layout as fp16 (the multi-index indirect-DMA path is not viable on
TRN2 HW, so the gather rides the same host permutation that shards the
term lists). Coords are pre-scaled per type (bond 1/8, angle 1/16,
torsion 1/32) so every fp16 intermediate stays in range; angle/torsion
formulas are scale-invariant, bond is compensated via K' = 64K,
x0' = x0/8.

Device (per core): fp16 DVE tensor ops run in the 2x packed perf mode;
squares / rsqrt (Abs_reciprocal_sqrt) / arctan run on the scalar (ACT)
engine so the two engines split the elementwise work. Torsion angle is
evaluated with the normalized triple-angle polynomial
  cos(3p - x0) = c(4c^2-3)cos(x0) + s(3-4s^2)sin(x0),  c = B/R, s = A/R
with B = n1.n2, A = -|b2| (b1.n2), avoiding atan2 entirely. Bond angle
theta = atan2(y, x) uses the half-angle form t = y/(r+|x|) in [0,1] so a
single ACT arctan (domain [-pi/2, pi/2]) suffices. Per-pose segment sums
are fused into the last DVE op of each term type via scalar_tensor_tensor
accum_out; the final cross-partition reduce is a ones-vector matmul on PE.
ACT table sets are grouped (abs_reciprocal_sqrt phase, then one switch to
the trig set for angle's arctan) so only one table load happens mid-run.
"""

import numpy as np

N_POSES = 64
MAX_ATOMS = 16384
N_CORES = 8
PP = N_POSES // N_CORES  # poses per core
P = 128
PI = float(np.pi)

SB = 1 / 8    # bond coord scale
SA = 1 / 16   # angle coord scale
ST = 1 / 32   # torsion coord scale
GB = 8        # poses per tile-group: bond
GA = 4        # angle
GT = 2        # torsion

_BUILD_CACHE = {}


# ----------------------------------------------------------------- host prep
def _bucket(atoms, param_idx, x0, K_table, arity):
    """Bucket terms by pose, pad to [N_POSES, arity, P, F] index tiles.

    Returns F, idx [N_POSES, arity, P, F] int32 (core-local flat atom row),
    K [N_POSES, P, F] f32 (0 on pads), x0 [N_POSES, P, F] f32.
    """
    n = atoms.shape[0]
    pose = (atoms[:, 0] // MAX_ATOMS).astype(np.int64)
    order = np.argsort(pose, kind="stable")
    pose_s = pose[order]
    atoms_s = atoms[order].astype(np.int64)
    x0_s = x0[order]
    K_s = K_table[param_idx[order]]

    counts = np.bincount(pose, minlength=N_POSES)
    F = -(-int(counts.max()) // P)  # ceil(max/P)
    F = -(-F // 4) * 4  # multiple of 4
    starts = np.zeros(N_POSES + 1, np.int64)
    np.cumsum(counts, out=starts[1:])
    r = np.arange(n, dtype=np.int64) - starts[pose_s]
    part = (r // F).astype(np.int64)
    free = (r % F).astype(np.int64)
    assert part.max() < P

    local = atoms_s - (pose_s * MAX_ATOMS)[:, None]
    corelocal = (local + ((pose_s % PP) * MAX_ATOMS)[:, None]).astype(np.int32)

    idx = np.zeros((N_POSES, arity, P, F), np.int32)
    idx[pose_s, :, part, free] = corelocal
    Kp = np.zeros((N_POSES, P, F), np.float32)
    Kp[pose_s, part, free] = K_s
    x0p = np.zeros((N_POSES, P, F), np.float32)
    x0p[pose_s, part, free] = x0_s
    return F, idx, Kp, x0p


def _gath16(ctab16, idx_core, G):
    """[PP, arity, P, F] idx + fp16 table -> [n_g, P, arity*3*G*F] fp16."""
    PPc, arity, Pp, F = idx_core.shape
    n_g = PPc // G
    g = ctab16[idx_core]  # [PP, arity, P, F, 3] fp16
    g = g.reshape(n_g, G, arity, Pp, F, 3).transpose(0, 3, 2, 5, 1, 4)
    return np.ascontiguousarray(g).reshape(n_g, Pp, arity * 3 * G * F)


def _prm16(arr, lo, hi, G):
    """[N_POSES, P, F] -> [n_g, P, G*F] fp16 for poses [lo, hi)."""
    a = arr[lo:hi].astype(np.float16)
    PPc, Pp, F = a.shape
    n_g = PPc // G
    a = a.reshape(n_g, G, Pp, F).transpose(0, 2, 1, 3)
    return np.ascontiguousarray(a).reshape(n_g, Pp, G * F)


# --------------------------------------------------------------- device build
def _build(Fb, Fa, Ft):
    key = (Fb, Fa, Ft)
    if key in _BUILD_CACHE:
        return _BUILD_CACHE[key]

    import concourse.bass as bass
    import concourse.tile as tile
    from concourse import bacc, mybir

    dt = mybir.dt
    f32 = dt.float32
    f16 = dt.float16
    Act = mybir.ActivationFunctionType
    Op = mybir.AluOpType

    nc = bacc.Bacc("TRN2", target_bir_lowering=False, debug=False,
                   num_devices=N_CORES)

    LB = GB * Fb   # bond free elems per pose-group (one group)
    LA = GA * Fa
    LT = GT * Ft
    NGA = PP // GA
    NGT = PP // GT

    bg_d = nc.dram_tensor("bg", [1, P, 2 * 3 * LB], f16, kind="ExternalInput").ap()
    bK_d = nc.dram_tensor("bK", [1, P, PP * Fb], f16, kind="ExternalInput").ap()
    bx_d = nc.dram_tensor("bx", [1, P, PP * Fb], f16, kind="ExternalInput").ap()
    ag_d = nc.dram_tensor("ag", [NGA, P, 3 * 3 * LA], f16, kind="ExternalInput").ap()
    aK_d = nc.dram_tensor("aK", [NGA, P, LA], f16, kind="ExternalInput").ap()
    ax_d = nc.dram_tensor("ax", [NGA, P, LA], f16, kind="ExternalInput").ap()
    tg_d = nc.dram_tensor("tg", [NGT, P, 4 * 3 * LT], f16, kind="ExternalInput").ap()
    tK_d = nc.dram_tensor("tK", [NGT, P, LT], f16, kind="ExternalInput").ap()
    tc_d = nc.dram_tensor("tc", [NGT, P, LT], f16, kind="ExternalInput").ap()
    ts_d = nc.dram_tensor("ts", [NGT, P, LT], f16, kind="ExternalInput").ap()
    out = nc.dram_tensor("out", [1, PP], f32, kind="ExternalOutput").ap()

    # Float biases for non-Copy activations resolve through const_aps keyed
    # (f32, value) — register the ones we use.
    for v in (1e-8, PI / 2):
        t = nc.alloc_sbuf_tensor(f"constf32-{v}", [P, 1], f32)
        nc.gpsimd.memset(t.ap(), v)
        nc.const_aps.aps[(f32, v)] = t.ap()
    nc.all_engine_barrier()

    from contextlib import ExitStack

    with tile.TileContext(nc) as tc, ExitStack() as ctx:
        pers = ctx.enter_context(tc.tile_pool(name="pers", bufs=1))
        gpool = ctx.enter_context(tc.tile_pool(name="g", bufs=2))
        wp = ctx.enter_context(tc.tile_pool(name="w", bufs=1))
        psum = ctx.enter_context(tc.tile_pool(name="ps", bufs=1, space="PSUM"))

        partials = pers.tile([P, PP * 3], f32)

        V = nc.vector
        S = nc.scalar

        def TT(o, a, b, op):
            V.tensor_tensor(out=o, in0=a, in1=b, op=op)

        # ---------------- persistent tiles (angle part1 -> part2) ---------
        at_t = pers.tile([P, PP * Fa], f16)    # angle t = y/(r+|x|)
        at_sg = pers.tile([P, PP * Fa], f16)   # angle sign(x)
        aK_t = pers.tile([P, PP * Fa], f16)
        ax_t = pers.tile([P, PP * Fa], f16)
        bK_t = pers.tile([P, PP * Fb], f16)
        bx_t = pers.tile([P, PP * Fb], f16)

        # =================== bond (one group of 8 poses) ===================
        bg = gpool.tile([P, 2 * 3 * LB], f16, tag="g", name="bg")
        nc.sync.dma_start(bg[:], bg_d[0])
        nc.sync.dma_start(bK_t[:], bK_d[0])
        nc.sync.dma_start(bx_t[:], bx_d[0])

        dv = wp.tile([P, 3 * LB], f16, tag="w3a", name="dv")
        TT(dv[:], bg[:, 0:3 * LB], bg[:, 3 * LB:6 * LB], Op.subtract)
        dsq = wp.tile([P, 3 * LB], f16, tag="w3b", name="dsq")
        S.activation(dsq[:], dv[:], Act.Square)
        D2 = wp.tile([P, LB], f16, tag="w1a", name="D2")
        TT(D2[:], dsq[:, 0:LB], dsq[:, LB:2 * LB], Op.add)
        TT(D2[:], D2[:], dsq[:, 2 * LB:3 * LB], Op.add)
        iD = wp.tile([P, LB], f16, tag="w1b", name="iD")
        S.activation(iD[:], D2[:], Act.Abs_reciprocal_sqrt, bias=1e-8)
        dd = wp.tile([P, LB], f16, tag="w1c", name="dd")
        TT(dd[:], D2[:], iD[:], Op.mult)
        TT(dd[:], dd[:], bx_t[:], Op.subtract)
        sqb = wp.tile([P, LB], f16, tag="w1d", name="sqb")
        S.activation(sqb[:], dd[:], Act.Square)
        e_b = wp.tile([P, Fb], f32, tag="we", name="e_b")
        for p in range(PP):
            sl = slice(p * Fb, (p + 1) * Fb)
            V.scalar_tensor_tensor(
                out=e_b[:], in0=sqb[:, sl], scalar=0.0, in1=bK_t[:, sl],
                op0=Op.add, op1=Op.mult,
                accum_out=partials[:, 3 * p:3 * p + 1])

        # =================== angle part 1 (through t, sign) ================
        for gi in range(NGA):
            ag = gpool.tile([P, 9 * LA], f16, tag="g", name="ag")
            nc.sync.dma_start(ag[:], ag_d[gi])
            psl = slice(gi * LA, (gi + 1) * LA)
            nc.sync.dma_start(aK_t[:, psl], aK_d[gi])
            nc.sync.dma_start(ax_t[:, psl], ax_d[gi])

            uv = wp.tile([P, 6 * LA], f16, tag="w6", name="uv")
            TT(uv[:, 0:3 * LA], ag[:, 0:3 * LA], ag[:, 3 * LA:6 * LA],
               Op.subtract)
            TT(uv[:, 3 * LA:6 * LA], ag[:, 6 * LA:9 * LA],
               ag[:, 3 * LA:6 * LA], Op.subtract)
            m3 = wp.tile([P, 3 * LA], f16, tag="w3a", name="m3")
            TT(m3[:], uv[:, 0:3 * LA], uv[:, 3 * LA:6 * LA], Op.mult)
            x = wp.tile([P, LA], f16, tag="w1a", name="x")
            TT(x[:], m3[:, 0:LA], m3[:, LA:2 * LA], Op.add)
            TT(x[:], x[:], m3[:, 2 * LA:3 * LA], Op.add)
            sq6 = wp.tile([P, 6 * LA], f16, tag="w6b", name="sq6")
            S.activation(sq6[:], uv[:], Act.Square)
            nu = wp.tile([P, LA], f16, tag="w1b", name="nu")
            TT(nu[:], sq6[:, 0:LA], sq6[:, LA:2 * LA], Op.add)
            TT(nu[:], nu[:], sq6[:, 2 * LA:3 * LA], Op.add)
            nv = wp.tile([P, LA], f16, tag="w1c", name="nv")
            TT(nv[:], sq6[:, 3 * LA:4 * LA], sq6[:, 4 * LA:5 * LA], Op.add)
            TT(nv[:], nv[:], sq6[:, 5 * LA:6 * LA], Op.add)
            Pn = wp.tile([P, LA], f16, tag="w1d", name="Pn")
            TT(Pn[:], nu[:], nv[:], Op.mult)
            x2 = wp.tile([P, LA], f16, tag="w1e", name="x2")
            S.activation(x2[:], x[:], Act.Square)
            Sc = wp.tile([P, LA], f16, tag="w1f", name="Sc")
            TT(Sc[:], Pn[:], x2[:], Op.subtract)
            iS = wp.tile([P, LA], f16, tag="w1g", name="iS")
            S.activation(iS[:], Sc[:], Act.Abs_reciprocal_sqrt, bias=1e-8)
            y = wp.tile([P, LA], f16, tag="w1h", name="y")
            TT(y[:], Sc[:], iS[:], Op.mult)
            iP = wp.tile([P, LA], f16, tag="w1i", name="iP")
            S.activation(iP[:], Pn[:], Act.Abs_reciprocal_sqrt, bias=1e-8)
            rr = wp.tile([P, LA], f16, tag="w1j", name="rr")
            TT(rr[:], Pn[:], iP[:], Op.mult)
            axv = wp.tile([P, LA], f16, tag="w1k", name="axv")
            S.activation(axv[:], x[:], Act.Abs)
            TT(rr[:], rr[:], axv[:], Op.add)  # den = r + |x|
            den2 = wp.tile([P, LA], f16, tag="w1l", name="den2")
            S.activation(den2[:], rr[:], Act.Square)
            ivd = wp.tile([P, LA], f16, tag="w1m", name="ivd")
            S.activation(ivd[:], den2[:], Act.Abs_reciprocal_sqrt, bias=1e-8)
            TT(at_t[:, psl], y[:], ivd[:], Op.mult)
            S.activation(at_sg[:, psl], x[:], Act.Sign)

        # =================== torsion (all groups) ==========================
        for gi in range(NGT):
            tg = gpool.tile([P, 12 * LT], f16, tag="g", name="tg")
            nc.sync.dma_start(tg[:], tg_d[gi])
            tK = gpool.tile([P, LT], f16, tag="tK", name="tK")
            nc.sync.dma_start(tK[:], tK_d[gi])
            tcx = gpool.tile([P, LT], f16, tag="tc", name="tcx")
            nc.sync.dma_start(tcx[:], tc_d[gi])
            tsx = gpool.tile([P, LT], f16, tag="ts", name="tsx")
            nc.sync.dma_start(tsx[:], ts_d[gi])

            # b_all = [b1|b2|b3] in one op
            b = wp.tile([P, 9 * LT], f16, tag="w9", name="b")
            TT(b[:], tg[:, 3 * LT:12 * LT], tg[:, 0:9 * LT], Op.subtract)
            b1 = [b[:, c * LT:(c + 1) * LT] for c in range(3)]
            b2 = [b[:, (3 + c) * LT:(4 + c) * LT] for c in range(3)]
            b3 = [b[:, (6 + c) * LT:(7 + c) * LT] for c in range(3)]

            def cross(pref, u, v, ta, tb, tn):
                res = wp.tile([P, 3 * LT], f16, tag=tn, name=pref)
                for c in range(3):
                    t1 = wp.tile([P, LT], f16, tag=ta, name="crA")
                    TT(t1[:], u[(c + 1) % 3], v[(c + 2) % 3], Op.mult)
                    t2 = wp.tile([P, LT], f16, tag=tb, name="crB")
                    TT(t2[:], u[(c + 2) % 3], v[(c + 1) % 3], Op.mult)
                    TT(res[:, c * LT:(c + 1) * LT], t1[:], t2[:], Op.subtract)
                return res

            n2 = cross("n2", b2, b3, "w1a", "w1b", "w3a")
            n1 = cross("n1", b1, b2, "w1a", "w1b", "w3b")

            def dot(pref, ut, vt, ta, tb):
                m = wp.tile([P, 3 * LT], f16, tag=ta, name="dm")
                TT(m[:], ut, vt, Op.mult)
                acc = wp.tile([P, LT], f16, tag=tb, name=pref)
                TT(acc[:], m[:, 0:LT], m[:, LT:2 * LT], Op.add)
                TT(acc[:], acc[:], m[:, 2 * LT:3 * LT], Op.add)
                return acc

            B = dot("B", n1[:], n2[:], "w3c", "w1c")
            dq = dot("dq", b[:, 0:3 * LT], n2[:], "w3c", "w1d")
            bsq = wp.tile([P, 3 * LT], f16, tag="w3c", name="bsq")
            S.activation(bsq[:], b[:, 3 * LT:6 * LT], Act.Square)
            S2 = wp.tile([P, LT], f16, tag="w1e", name="S2")
            TT(S2[:], bsq[:, 0:LT], bsq[:, LT:2 * LT], Op.add)
            TT(S2[:], S2[:], bsq[:, 2 * LT:3 * LT], Op.add)
            d2 = wp.tile([P, LT], f16, tag="w1f", name="d2")
            S.activation(d2[:], dq[:], Act.Square)
            A2 = wp.tile([P, LT], f16, tag="w1g", name="A2")
            TT(A2[:], S2[:], d2[:], Op.mult)
            B2 = wp.tile([P, LT], f16, tag="w1h", name="B2")
            S.activation(B2[:], B[:], Act.Square)
            R2 = wp.tile([P, LT], f16, tag="w1i", name="R2")
            TT(R2[:], A2[:], B2[:], Op.add)
            iR = wp.tile([P, LT], f16, tag="w1j", name="iR")
            S.activation(iR[:], R2[:], Act.Abs_reciprocal_sqrt, bias=1e-8)
            cc = wp.tile([P, LT], f16, tag="w1k", name="cc")
            TT(cc[:], B[:], iR[:], Op.mult)
            iS2 = wp.tile([P, LT], f16, tag="w1l", name="iS2")
            S.activation(iS2[:], S2[:], Act.Abs_reciprocal_sqrt, bias=1e-8)
            hh = wp.tile([P, LT], f16, tag="w1m", name="hh")
            TT(hh[:], S2[:], iS2[:], Op.mult)
            TT(hh[:], hh[:], dq[:], Op.mult)        # h*d
            sm = wp.tile([P, LT], f16, tag="w1n", name="sm")
            TT(sm[:], hh[:], iR[:], Op.mult)
            c2 = wp.tile([P, LT], f16, tag="w1o", name="c2")
            S.activation(c2[:], cc[:], Act.Square)
            w1 = wp.tile([P, LT], f16, tag="w1p", name="w1")
            V.tensor_scalar(out=w1[:], in0=c2[:], scalar1=4.0, scalar2=-3.0,
                            op0=Op.mult, op1=Op.add)
            cos3 = wp.tile([P, LT], f16, tag="w1q", name="cos3")
            TT(cos3[:], cc[:], w1[:], Op.mult)
            s2 = wp.tile([P, LT], f16, tag="w1o", name="s2")
            S.activation(s2[:], sm[:], Act.Square)
            w2 = wp.tile([P, LT], f16, tag="w1p", name="w2")
            V.tensor_scalar(out=w2[:], in0=s2[:], scalar1=-4.0, scalar2=3.0,
                            op0=Op.mult, op1=Op.add)
            sin3 = wp.tile([P, LT], f16, tag="w1r", name="sin3")
            TT(sin3[:], sm[:], w2[:], Op.mult)
            TT(cos3[:], cos3[:], tcx[:], Op.mult)   # qa
            TT(sin3[:], sin3[:], tsx[:], Op.mult)   # qb
            q = wp.tile([P, LT], f16, tag="w1s", name="q")
            TT(q[:], cos3[:], sin3[:], Op.add)
            e_t = wp.tile([P, Ft], f16, tag="we", name="e_t")
            for p in range(GT):
                pose = gi * GT + p
                sl = slice(p * Ft, (p + 1) * Ft)
                V.scalar_tensor_tensor(
                    out=e_t[:], in0=q[:, sl], scalar=1.0, in1=tK[:, sl],
                    op0=Op.add, op1=Op.mult,
                    accum_out=partials[:, 3 * pose + 2:3 * pose + 3])

        # =================== angle part 2 (trig table set) =================
        for gi in range(NGA):
            psl = slice(gi * LA, (gi + 1) * LA)
            ph = wp.tile([P, LA], f16, tag="w1a", name="ph")
            S.activation(ph[:], at_t[:, psl], Act.Arctan)
            qq = wp.tile([P, LA], f16, tag="w1b", name="qq")
            V.tensor_scalar(out=qq[:], in0=ph[:], scalar1=2.0,
                            scalar2=-PI / 2, op0=Op.mult, op1=Op.add)
            TT(qq[:], at_sg[:, psl], qq[:], Op.mult)
            TT(qq[:], qq[:], ax_t[:, psl], Op.subtract)
            sqa = wp.tile([P, LA], f16, tag="w1c", name="sqa")
            S.activation(sqa[:], qq[:], Act.Square, bias=PI / 2)
            e_a = wp.tile([P, Fa], f16, tag="we", name="e_a")
            for p in range(GA):
                pose = gi * GA + p
                sl = slice(p * Fa, (p + 1) * Fa)
                V.scalar_tensor_tensor(
                    out=e_a[:], in0=sqa[:, sl], scalar=0.0,
                    in1=aK_t[:, pose * Fa:(pose + 1) * Fa],
                    op0=Op.add, op1=Op.mult,
                    accum_out=partials[:, 3 * pose + 1:3 * pose + 2])

        # =================== final cross-partition reduce ==================
        ones = pers.tile([P, 1], f32)
        V.memset(ones[:], 1.0)
        ps = psum.tile([1, PP * 3], f32)
        nc.tensor.matmul(out=ps[:], lhsT=ones[:], rhs=partials[:],
                         start=True, stop=True)
        psc = pers.tile([1, PP * 3], f32)
        V.tensor_copy(out=psc[:], in_=ps[:])
        s8 = pers.tile([1, PP], f32)
        V.tensor_tensor(out=s8[:], in0=psc[0:1, 0:PP * 3:3],
                        in1=psc[0:1, 1:PP * 3:3], op=Op.add)
        V.tensor_tensor(out=s8[:], in0=s8[:], in1=psc[0:1, 2:PP * 3:3],
                        op=Op.add)
        nc.sync.dma_start(out[:], s8[:])

    nc.compile()
    _BUILD_CACHE[key] = nc
    return nc


# ---------------------------------------------------------------------- main
def kernel(coords, global_params, bond_x0, angle_x0, tor_x0,
           bond_atoms, bond_param_idx, angle_atoms, angle_param_idx,
           tor_atoms, tor_param_idx, _trace=False):
    coords = np.asarray(coords, dtype=np.float32)
    K_table = np.asarray(global_params, dtype=np.float32)[:, 0]

    Fb, bidx, bK, bx0 = _bucket(np.asarray(bond_atoms),
                                np.asarray(bond_param_idx),
                                np.asarray(bond_x0, np.float32), K_table, 2)
    Fa, aidx, aK, ax0 = _bucket(np.asarray(angle_atoms),
                                np.asarray(angle_param_idx),
                                np.asarray(angle_x0, np.float32), K_table, 3)
    Ft, tidx, tK, tx0 = _bucket(np.asarray(tor_atoms),
                                np.asarray(tor_param_idx),
                                np.asarray(tor_x0, np.float32), K_table, 4)

    nc = _build(Fb, Fa, Ft)

    # bond scale compensation; torsion ships cos(x0) / -sin(x0)
    bKs = bK * 64.0
    bx0s = bx0 * SB
    tcx = np.cos(tx0)
    tsxn = -np.sin(tx0)

    flat = coords.reshape(N_CORES, PP * MAX_ATOMS, 3)
    in_maps = []
    for c in range(N_CORES):
        lo, hi = c * PP, (c + 1) * PP
        ctb = (flat[c] * SB).astype(np.float16)
        cta = (flat[c] * SA).astype(np.float16)
        ctt = (flat[c] * ST).astype(np.float16)
        in_maps.append({
            "bg": _gath16(ctb, bidx[lo:hi], GB),
            "bK": _prm16(bKs, lo, hi, GB),
            "bx": _prm16(bx0s, lo, hi, GB),
            "ag": _gath16(cta, aidx[lo:hi], GA),
            "aK": _prm16(aK, lo, hi, GA),
            "ax": _prm16(ax0, lo, hi, GA),
            "tg": _gath16(ctt, tidx[lo:hi], GT),
            "tK": _prm16(tK, lo, hi, GT),
            "tc": _prm16(tcx, lo, hi, GT),
            "ts": _prm16(tsxn, lo, hi, GT),
        })

    from concourse.bass_utils import run_bass_kernel_spmd
    res = run_bass_kernel_spmd(nc, in_maps, list(range(N_CORES)),
                               trace=_trace)
    out = np.concatenate([res.results[c]["out"][0] for c in range(N_CORES)])
    if _trace:
        kernel._last_result = res
    return out.astype(np.float32)
